# revision 2
# baseline (speedup 1.0000x reference)
"""Trainium2 Bass kernel for nn_AttentionLayer (topk_masking).

Computes, for full inputs feat_x [N,D], feat_y [M,D], W1 [D,D], k=32:
    e = (feat_x @ W1) @ feat_y.T / (|x_i| |y_j| + eps)
    A = scatter(top_k(e, k)) into -9e15 filled matrix; softmax rows
Output attention [N, M] f32.  Rows are sharded across 8 NeuronCores
(row-parallel, feat_y/W1 replicated); all compute is core-local.

Per-core algorithm (rows_shard x M):
  - normalize x rows by 1/|x|, y rows by 1/|y| (eps is negligible vs |x||y|~256)
  - transpose via PE matmul-with-identity -> XnT [D, rows], VnT [D, M]
  - UnT = (Xn @ W1)^T via PE
  - per 128-row tile: e = UnT.T @ VnT in PSUM chunks -> SBUF
  - exact top-32 per row: 4 rounds of vector.max (top-8) + match_replace
  - tau = 32nd value, m = max, Z = sum exp(v-m) over the 32 values
  - out = (e >= tau) * exp(e - m - lnZ), DMA'd to DRAM
The softmax of the reference over the -9e15-filled matrix is exactly 0 off
the top-k positions (exp underflow), so this reproduces it.
"""

import os
import sys

import numpy as np

_TRN_REPO = "/opt/trn_rl_repo"
if _TRN_REPO not in sys.path:
    sys.path.insert(0, _TRN_REPO)

import concourse.bass as bass  # noqa: E402
import concourse.mybir as mybir  # noqa: E402
from concourse import bacc  # noqa: E402
from concourse.tile import TileContext  # noqa: E402
from concourse.bass_utils import run_bass_kernel_spmd  # noqa: E402
from concourse.masks import make_identity  # noqa: E402

P = 128
F32 = mybir.dt.float32
AL = mybir.AluOpType
AF = mybir.ActivationFunctionType

N_CORES = 8
N_FULL = 8192
M_FULL = 8192
D_FULL = 256
K_TOP = 32
NEG_FILL = -1.0e30


def _body(tc, out, fx, fy, w1, n_rows, m_cols, d):
    nc = tc.nc
    NT_X = n_rows // P          # x row-tiles in this shard
    NT_Y = m_cols // P          # y row-tiles
    NH = d // P                 # halves of the contraction dim
    NCH = m_cols // 512         # 512-col psum chunks per x-tile
    RC = n_rows // 512 if n_rows >= 512 else 1
    RCW = min(n_rows, 512)

    import contextlib
    with contextlib.ExitStack() as ctx:
        pp = ctx.enter_context(tc.tile_pool(name="persist", bufs=1))
        vnT = pp.tile([P, NH, m_cols], F32)
        unT = pp.tile([P, NH, n_rows], F32)
        w_sb = pp.tile([P, NH, d], F32)
        ident = pp.tile([P, P], F32)

        make_identity(nc, ident)
        nc.sync.dma_start(w_sb, w1.rearrange("(h p) e -> p h e", p=P))

        # ---------------- X side: norms, normalize, transpose, UnT ------
        with tc.tile_pool(name="prox", bufs=1) as px, \
             tc.tile_pool(name="proxp", bufs=4, space="PSUM") as pxp:
            x_all = px.tile([P, NT_X, d], F32)
            nc.sync.dma_start(x_all, fx.rearrange("(t p) e -> p t e", p=P))
            xsq = px.tile([P, NT_X, d], F32)
            nc.scalar.square(xsq, x_all)
            n1sq = px.tile([P, NT_X], F32)
            nc.vector.reduce_sum(n1sq, xsq, axis=mybir.AxisListType.X)
            n1 = px.tile([P, NT_X], F32)
            nc.scalar.sqrt(n1, n1sq)
            n1r = px.tile([P, NT_X], F32)
            nc.vector.reciprocal(n1r, n1)

            xnT = px.tile([P, NH, n_rows], F32)
            for tx in range(NT_X):
                xs = px.tile([P, d], F32, tag="xscaled")
                nc.vector.tensor_scalar(
                    xs, x_all[:, tx, :], n1r[:, tx:tx + 1],
                    scalar2=None, op0=AL.mult)
                for h in range(NH):
                    ps = pxp.tile([P, P], F32, tag="psx")
                    nc.tensor.matmul(ps, lhsT=xs[:, h * P:(h + 1) * P],
                                     rhs=ident, start=True, stop=True)
                    nc.vector.tensor_copy(xnT[:, h, tx * P:(tx + 1) * P], ps)

            for h2 in range(NH):
                for rc in range(RC):
                    psu = pxp.tile([P, RCW], F32, tag="psu")
                    for h1 in range(NH):
                        nc.tensor.matmul(
                            psu,
                            lhsT=w_sb[:, h1, h2 * P:(h2 + 1) * P],
                            rhs=xnT[:, h1, rc * RCW:(rc + 1) * RCW],
                            start=(h1 == 0), stop=(h1 == NH - 1))
                    nc.vector.tensor_copy(
                        unT[:, h2, rc * RCW:(rc + 1) * RCW], psu)

        # ---------------- Y side: norms, normalize, transpose -> VnT ----
        YG = 16 if NT_Y % 16 == 0 else NT_Y   # y tiles per group
        with tc.tile_pool(name="proy", bufs=2) as py, \
             tc.tile_pool(name="proyp", bufs=4, space="PSUM") as pyp:
            for q in range(NT_Y // YG):
                yq = py.tile([P, YG, d], F32, tag="yq")
                nc.sync.dma_start(
                    yq,
                    fy[q * YG * P:(q + 1) * YG * P, :]
                    .rearrange("(t p) e -> p t e", p=P))
                ysq = py.tile([P, YG, d], F32, tag="ysq")
                nc.scalar.square(ysq, yq)
                n2sq = py.tile([P, YG], F32, tag="n2sq")
                nc.vector.reduce_sum(n2sq, ysq, axis=mybir.AxisListType.X)
                n2 = py.tile([P, YG], F32, tag="n2")
                nc.scalar.sqrt(n2, n2sq)
                n2r = py.tile([P, YG], F32, tag="n2r")
                nc.vector.reciprocal(n2r, n2)

                GB = 4 if YG % 4 == 0 else 1   # batch 4 tiles per psum bank
                for g in range(YG // GB):
                    psg = [pyp.tile([P, GB * P], F32, tag=f"psy{h}",
                                    name=f"psy{h}")
                           for h in range(NH)]
                    for j in range(GB):
                        tyl = g * GB + j
                        ysc = py.tile([P, d], F32, tag="yscaled")
                        nc.vector.tensor_scalar(
                            ysc, yq[:, tyl, :], n2r[:, tyl:tyl + 1],
                            scalar2=None, op0=AL.mult)
                        for h in range(NH):
                            nc.tensor.matmul(
                                psg[h][:, j * P:(j + 1) * P],
                                lhsT=ysc[:, h * P:(h + 1) * P],
                                rhs=ident, start=True, stop=True)
                    col0 = (q * YG + g * GB) * P
                    for h in range(NH):
                        nc.vector.tensor_copy(
                            vnT[:, h, col0:col0 + GB * P], psg[h])

        # ---------------- main loop over x row-tiles --------------------
        with tc.tile_pool(name="main", bufs=1) as mp, \
             tc.tile_pool(name="small", bufs=2) as sp, \
             tc.tile_pool(name="mpsum", bufs=4, space="PSUM") as msp:
            for tx in range(NT_X):
                e_sb = mp.tile([P, m_cols], F32, tag="e")
                t_sb = mp.tile([P, m_cols], F32, tag="t")
                o_sb = mp.tile([P, m_cols], F32, tag="o")
                for cn in range(NCH):
                    pe = msp.tile([P, 512], F32, tag="pe")
                    for h in range(NH):
                        nc.tensor.matmul(
                            pe,
                            lhsT=unT[:, h, tx * P:(tx + 1) * P],
                            rhs=vnT[:, h, cn * 512:(cn + 1) * 512],
                            start=(h == 0), stop=(h == NH - 1))
                    if cn % 2 == 0:
                        nc.vector.tensor_copy(e_sb[:, cn * 512:(cn + 1) * 512], pe)
                    else:
                        nc.scalar.copy(e_sb[:, cn * 512:(cn + 1) * 512], pe)

                # exact top-32 ladder (destroys o_sb scratch copy of e)
                v32 = sp.tile([P, K_TOP], F32, tag="v32")
                nc.vector.tensor_copy(o_sb, e_sb)
                for r in range(K_TOP // 8):
                    nc.vector.max(out=v32[:, 8 * r:8 * r + 8], in_=o_sb)
                    nc.vector.match_replace(
                        out=o_sb, in_to_replace=v32[:, 8 * r:8 * r + 8],
                        in_values=o_sb, imm_value=NEG_FILL)

                negm = sp.tile([P, 1], F32, tag="negm")
                nc.vector.tensor_scalar(negm, v32[:, 0:1], -1.0,
                                        scalar2=None, op0=AL.mult)
                e32 = sp.tile([P, K_TOP], F32, tag="e32")
                zsum = sp.tile([P, 1], F32, tag="zsum")
                nc.scalar.activation(e32, v32, AF.Exp, bias=negm,
                                     accum_out=zsum)
                lnz = sp.tile([P, 1], F32, tag="lnz")
                nc.scalar.activation(lnz, zsum, AF.Ln)
                bias2 = sp.tile([P, 1], F32, tag="bias2")
                nc.vector.scalar_tensor_tensor(
                    out=bias2, in0=v32[:, 0:1], scalar=-1.0, in1=lnz,
                    op0=AL.mult, op1=AL.subtract)
                nc.scalar.activation(t_sb, e_sb, AF.Exp, bias=bias2)
                nc.vector.scalar_tensor_tensor(
                    out=o_sb, in0=e_sb, scalar=v32[:, 31:32], in1=t_sb,
                    op0=AL.is_ge, op1=AL.mult)
                nc.sync.dma_start(out[tx * P:(tx + 1) * P, :], o_sb)


def build(n_rows=N_FULL // N_CORES, m_cols=M_FULL, d=D_FULL,
          n_cores=N_CORES):
    nc = bacc.Bacc("TRN2", target_bir_lowering=False, debug=False,
                   enable_asserts=False, num_devices=n_cores)
    fx = nc.dram_tensor("feat_x", [n_rows, d], F32, kind="ExternalInput").ap()
    fy = nc.dram_tensor("feat_y", [m_cols, d], F32, kind="ExternalInput").ap()
    w1 = nc.dram_tensor("W1", [d, d], F32, kind="ExternalInput").ap()
    out = nc.dram_tensor("out", [n_rows, m_cols], F32,
                         kind="ExternalOutput").ap()
    with TileContext(nc) as tc:
        _body(tc, out, fx, fy, w1, n_rows, m_cols, d)
    nc.compile()
    return nc


_CACHE = {}


def _get_nc():
    if "nc" not in _CACHE:
        _CACHE["nc"] = build()
    return _CACHE["nc"]


def make_in_maps(feat_x, feat_y, W1):
    feat_x = np.ascontiguousarray(feat_x, dtype=np.float32)
    feat_y = np.ascontiguousarray(feat_y, dtype=np.float32)
    W1 = np.ascontiguousarray(W1, dtype=np.float32)
    shard = feat_x.shape[0] // N_CORES
    return [
        {"feat_x": feat_x[c * shard:(c + 1) * shard],
         "feat_y": feat_y, "W1": W1}
        for c in range(N_CORES)
    ]


def kernel(feat_x, feat_y, W1, k):
    assert int(k) == K_TOP, f"kernel compiled for k=32, got {k}"
    assert feat_x.shape == (N_FULL, D_FULL)
    assert feat_y.shape == (M_FULL, D_FULL)
    nc = _get_nc()
    in_maps = make_in_maps(feat_x, feat_y, W1)
    res = run_bass_kernel_spmd(nc, in_maps, core_ids=list(range(N_CORES)))
    outs = [res.results[c]["out"] for c in range(N_CORES)]
    return np.concatenate(outs, axis=0).astype(np.float32)


if __name__ == "__main__":
    rng = np.random.default_rng(0)
    fx = rng.standard_normal((N_FULL, D_FULL), dtype=np.float32)
    fy = rng.standard_normal((M_FULL, D_FULL), dtype=np.float32)
    limit = np.sqrt(6.0 / (D_FULL + D_FULL))
    w = rng.uniform(-limit, limit, (D_FULL, D_FULL)).astype(np.float32)
    o = kernel(fx, fy, w, 32)
    print(o.shape, o.dtype, float(o.sum()))
